# revision 1
# baseline (speedup 1.0000x reference)
"""BrainRNN forward pass on 8 TRN2 NeuronCores (Bass/Tile, SPMD).

Strategy (tensor-parallel over output neurons, fp32 exact):
  - Each block's 1024 output neurons are row-sharded 128/core; masks are
    folded into weights on the host (memory-bound: halves HBM traffic).
  - Matmuls run with the (streamed) weights as the MOVING operand and the
    activations as the 64-wide stationary operand: psum[b, m] += sum_k
    actT[k, b] * W.T[k, m].  fp32 moving runs ~2 cyc/row vs ~4x slower
    weights-stationary LDWEIGHTS.
  - Adjacent blocks share one [64, 256] PSUM accumulator (bank budget);
    per-block biases enter via a K=1 "ones x biasrow" matmul that opens
    each accumulator.
  - Chain per block: sigmoid (PSUM->SBUF), PE transpose to [128(m), 64(b)],
    DVE copy, gpsimd bounce DMA, AllGather over 8 cores, 3-way staggered
    unpack on the scalar ring.
  - DMA emission order tracks the chain's deadlines (rec block j due at
    sigmoid_j); matmul emission back-fills AllGather wait windows with
    ready work (later-pair skip contributions) to keep the PE warm.
  - Output block: each core contracts its own 128 rows of cur7 against
    W_out[:, rows].T (N=512), then one ReduceScatter yields each core an
    8-row batch shard of the [64, 512] output.
"""

import numpy as np

N = 8192
W = 1024
L = 8
B = 64
IN = 512
OUT = 512
NCORES = 8
RP = W // NCORES        # 128 rows per core per block

_BUILT = None


def _pack(A):
    """[M, K] -> [128, (K/128)*M] with packed[p, k*M+m] = A[m, k*128+p].

    Chunk kidx is A[:, kidx*128:(kidx+1)*128].T, i.e. [K=128(part), M(free)].
    """
    M, K = A.shape
    nk = K // 128
    return np.ascontiguousarray(
        A.reshape(M, nk, 128).transpose(2, 1, 0).reshape(128, nk * M)
    )


def _interleave(Pa, Pb):
    """Two packed [128, nk*128] -> [128, nk*256] with per-chunk interleave."""
    nk = Pa.shape[1] // 128
    out = np.empty((128, nk, 2, 128), np.float32)
    out[:, :, 0, :] = Pa.reshape(128, nk, 128)
    out[:, :, 1, :] = Pb.reshape(128, nk, 128)
    return np.ascontiguousarray(out.reshape(128, nk * 256))


def _build():
    import concourse.bass as bass
    import concourse.bacc as bacc
    import concourse.mybir as mybir
    import concourse.tile as tile

    fp32 = mybir.dt.float32
    AF = mybir.ActivationFunctionType

    nc = bacc.Bacc(
        "TRN2",
        target_bir_lowering=False,
        debug=False,
        enable_asserts=False,
        num_devices=NCORES,
    )

    t_hT = nc.dram_tensor("hT", [128, 64 * B], fp32, kind="ExternalInput")
    t_xT = nc.dram_tensor("xT", [128, 4 * B], fp32, kind="ExternalInput")
    t_win = nc.dram_tensor("win", [128, 4 * RP], fp32, kind="ExternalInput")
    t_rec = {
        j: nc.dram_tensor(f"rec{j}", [128, 64 * 128], fp32, kind="ExternalInput")
        for j in range(7)
    }
    t_hid = nc.dram_tensor("hid", [7, 128, 8 * RP], fp32, kind="ExternalInput")
    # skip{q}_{c}: pair q=(2q, 2q+1), cur-source block c.  c < 2q: both
    # blocks interleaved [128, 8*256]; c == 2q: later block only [128, 8*128].
    t_skip = {}
    for q in range(1, 4):
        a = 2 * q
        for c in range(a + 1):
            wdt = 8 * 256 if c < a else 8 * 128
            t_skip[(q, c)] = nc.dram_tensor(
                f"skip{q}_{c}", [128, wdt], fp32, kind="ExternalInput"
            )
    t_biasrow = nc.dram_tensor("biasrow", [1, 4 * 256], fp32, kind="ExternalInput")
    t_wout = nc.dram_tensor("wout", [128, 512], fp32, kind="ExternalInput")
    t_boutrow = nc.dram_tensor("boutrow", [1, 512], fp32, kind="ExternalInput")
    t_ones = nc.dram_tensor("ones", [1, B], fp32, kind="ExternalInput")
    t_ident = nc.dram_tensor("ident", [B, B], fp32, kind="ExternalInput")
    t_out = nc.dram_tensor("out", [8, 512], fp32, kind="ExternalOutput")

    rg = [list(range(NCORES))]
    qof = lambda j: j // 2          # pair index of block j
    side = lambda j: j % 2          # column side within pair tile

    with tile.TileContext(nc) as tc:
        with (
            tc.tile_pool(name="const", bufs=1) as constp,
            tc.tile_pool(name="wrec", bufs=3) as wrecp,
            tc.tile_pool(name="whid", bufs=1) as whidp,
            tc.tile_pool(name="wskip", bufs=1) as wskipp,
            tc.tile_pool(name="curs", bufs=1) as curp,
            tc.tile_pool(name="psum", bufs=1, space="PSUM") as psump,
            tc.tile_pool(name="dram", bufs=3, space="DRAM") as dramp,
        ):
            # ---- persistent inputs -------------------------------------
            hT_sb = constp.tile([128, 64 * B], fp32, name="hT_sb", tag="hT")
            nc.sync.dma_start(out=hT_sb, in_=t_hT[:, :])
            xT_sb = constp.tile([128, 4 * B], fp32, name="xT_sb", tag="xT")
            nc.sync.dma_start(out=xT_sb, in_=t_xT[:, :])
            win_sb = constp.tile([128, 4 * RP], fp32, name="win_sb", tag="win")
            nc.sync.dma_start(out=win_sb, in_=t_win[:, :])
            biasrow_sb = constp.tile([1, 4 * 256], fp32, name="biasrow_sb", tag="br")
            nc.sync.dma_start(out=biasrow_sb, in_=t_biasrow[:, :])
            wout_sb = constp.tile([128, 512], fp32, name="wout_sb", tag="wout")
            nc.sync.dma_start(out=wout_sb, in_=t_wout[:, :])
            boutrow_sb = constp.tile([1, 512], fp32, name="boutrow_sb", tag="bo")
            nc.sync.dma_start(out=boutrow_sb, in_=t_boutrow[:, :])
            ones_sb = constp.tile([1, B], fp32, name="ones_sb", tag="ones")
            nc.sync.dma_start(out=ones_sb, in_=t_ones[:, :])
            ident_sb = constp.tile([B, B], fp32, name="ident_sb", tag="ident")
            nc.sync.dma_start(out=ident_sb, in_=t_ident[:, :])

            psA = [
                psump.tile([64, 256], fp32, name=f"psA{q}", tag=f"psA{q}")
                for q in range(4)
            ]
            curT = [None] * 8

            # bias-init: psA[q] = ones.T @ biasrow[q]  (start=True opens group)
            for q in range(4):
                nc.tensor.matmul(
                    psA[q],
                    lhsT=ones_sb[:, :],
                    rhs=biasrow_sb[:, q * 256:(q + 1) * 256],
                    start=True,
                    stop=False,
                )

            def rec_blk(j):
                """Stream block j's rec weights (1MB tiles, 512KB DMAs)."""
                q, s = qof(j), side(j)
                for h in range(4):
                    rt = wrecp.tile([128, 2048], fp32, name=f"rec{j}h{h}", tag="rec")
                    for i in range(2):
                        nc.sync.dma_start(
                            out=rt[:, i * 1024:(i + 1) * 1024],
                            in_=t_rec[j][:, h * 2048 + i * 1024:
                                         h * 2048 + (i + 1) * 1024],
                        )
                    for k in range(16):
                        kg = h * 16 + k
                        nc.tensor.matmul(
                            psA[q][:, s * 128:(s + 1) * 128],
                            lhsT=hT_sb[:, kg * B:(kg + 1) * B],
                            rhs=rt[:, k * 128:(k + 1) * 128],
                            start=False,
                            stop=False,
                        )

            def chain_tail(j):
                """sigmoid -> transpose -> AllGather -> staggered unpack."""
                q, s = qof(j), side(j)
                cp = curp.tile([64, 128], fp32, name=f"cpart{j}", tag="cpart", bufs=2)
                nc.scalar.activation(cp, psA[q][:, s * 128:(s + 1) * 128], AF.Sigmoid)
                pt = psump.tile([128, B], fp32, name=f"pt{j}", tag="pt", bufs=2)
                nc.tensor.transpose(pt, cp, ident_sb[:, :])
                ptsb = curp.tile([128, B], fp32, name=f"ptsb{j}", tag="ptsb", bufs=2)
                nc.vector.tensor_copy(ptsb, pt)
                agin = dramp.tile([128, B], fp32, name=f"agin{j}", tag="agin")
                agout = dramp.tile([W, B], fp32, name=f"agout{j}", tag="agout")
                nc.gpsimd.dma_start(out=agin, in_=ptsb)
                nc.gpsimd.collective_compute(
                    "AllGather",
                    mybir.AluOpType.bypass,
                    replica_groups=rg,
                    ins=[agin.opt()],
                    outs=[agout.opt()],
                )
                dst = curp.tile([128, 8 * B], fp32, name=f"curT{j}", tag="curT",
                                bufs=6)
                # gpsimd just observed the collective's completion sem: it can
                # issue chunks 0-3 with no cross-engine hop; scalar does 4-7
                # in parallel on its own ring.
                for eng, kk in ((nc.gpsimd, 0), (nc.scalar, 4)):
                    eng.dma_start(
                        out=dst[:, kk * B:(kk + 4) * B].rearrange(
                            "p (k b) -> p k b", k=4
                        ),
                        in_=agout[kk * 128:(kk + 4) * 128, :].rearrange(
                            "(k p) b -> p k b", p=128
                        ),
                    )
                curT[j] = dst

            hid_tiles = {}

            def hid_tile(j):
                # resident (distinct tag): allocation can never stall the ring
                ht = whidp.tile([128, 8 * RP], fp32, name=f"hid{j}", tag=f"hid{j}")
                nc.sync.dma_start(out=ht, in_=t_hid[j - 1, :, :])
                hid_tiles[j] = ht

            def hid_mms(j):
                """cur_{j-1} @ W_hid[j-1].T into block j's psum columns."""
                q, s = qof(j), side(j)
                if j not in hid_tiles:
                    hid_tile(j)
                ht = hid_tiles[j]
                for kk in range(8):
                    nc.tensor.matmul(
                        psA[q][:, s * 128:(s + 1) * 128],
                        lhsT=curT[j - 1][:, kk * B:(kk + 1) * B],
                        rhs=ht[:, kk * RP:(kk + 1) * RP],
                        start=False,
                        stop=(s == 1 and kk == 7),
                    )

            skip_tiles = {}

            def skip_tile(q, c):
                """Allocate + DMA pair q's skip weights sourced from cur_c."""
                a = 2 * q
                wide = 256 if c < a else 128
                st = wskipp.tile(
                    [128, 8 * wide], fp32, name=f"skip{q}_{c}t", tag=f"skip{q}_{c}"
                )
                half = 4 * wide
                for i in range(2):
                    nc.sync.dma_start(
                        out=st[:, i * half:(i + 1) * half],
                        in_=t_skip[(q, c)][:, i * half:(i + 1) * half],
                    )
                skip_tiles[(q, c)] = st

            def skip_one(q, c, part=None):
                """MMs for pair q's skip from cur_c.  part: None=all columns,
                'lo'=first block's 128 cols, 'hi'=second block's 128 cols."""
                a = 2 * q
                wide = 256 if c < a else 128
                if (q, c) not in skip_tiles:
                    skip_tile(q, c)
                st = skip_tiles[(q, c)]
                off = 0 if c < a else 128
                rlo, rwide = 0, wide
                if part == "lo" and wide == 256:
                    rlo, rwide = 0, 128
                elif part == "hi" and wide == 256:
                    rlo, rwide = 128, 128
                for kk in range(8):
                    nc.tensor.matmul(
                        psA[q][:, off + rlo:off + rlo + rwide],
                        lhsT=curT[c][:, kk * B:(kk + 1) * B],
                        rhs=st[:, kk * wide + rlo:kk * wide + rlo + rwide],
                        start=False,
                        stop=False,
                    )

            # ---- block 0: x @ W_in.T + h @ Wrec(block0).T (+bias) ------
            for kk in range(4):
                nc.tensor.matmul(
                    psA[0][:, 0:128],
                    lhsT=xT_sb[:, kk * B:(kk + 1) * B],
                    rhs=win_sb[:, kk * RP:(kk + 1) * RP],
                    start=False,
                    stop=False,
                )
            rec_blk(0)
            chain_tail(0)

            hid_tile(1)
            rec_blk(1)                  # AG_0 window work
            hid_mms(1)
            chain_tail(1)

            skip_tile(1, 1)
            skip_tile(1, 0)
            hid_tile(2)
            rec_blk(2)                  # AG_1 window work
            skip_one(1, 0)
            skip_one(1, 1, part="lo")   # urgent: block 2's last skip source
            hid_mms(2)
            chain_tail(2)

            skip_tile(1, 2)
            hid_tile(3)
            skip_tile(2, 0)
            rec_blk(3)                  # AG_2 window work
            skip_one(1, 1, part="hi")
            skip_one(1, 2)              # urgent for block 3
            hid_mms(3)
            chain_tail(3)

            rec_blk(4)                  # AG_3 window work
            skip_tile(2, 1)
            hid_tile(4)
            skip_tile(2, 2)
            skip_tile(2, 3)
            skip_one(2, 0)
            skip_one(2, 1)
            skip_one(2, 2)
            skip_one(2, 3, part="lo")   # urgent for block 4
            hid_mms(4)
            chain_tail(4)

            rec_blk(5)                  # AG_4 window work
            hid_tile(5)
            skip_tile(3, 0)
            skip_tile(2, 4)
            skip_one(3, 0)
            skip_one(2, 3, part="hi")
            skip_one(2, 4)              # urgent for block 5
            hid_mms(5)
            chain_tail(5)

            rec_blk(6)                  # AG_5 window work
            skip_tile(3, 1)
            skip_tile(3, 2)
            skip_tile(3, 3)
            skip_tile(3, 4)
            skip_tile(3, 5)
            skip_one(3, 1)
            skip_one(3, 2)
            skip_one(3, 3)
            skip_one(3, 4)
            skip_one(3, 5, part="lo")   # urgent for block 6
            hid_mms(6)
            chain_tail(6)

            skip_tile(3, 6)
            hid_tile(7)
            skip_one(3, 5, part="hi")   # AG_6 window work
            skip_one(3, 6)              # urgent for block 7
            hid_mms(7)

            # block 7 tail: sigmoid -> transpose -> local out matmul -> RS
            cp7 = curp.tile([64, 128], fp32, name="cpart7", tag="cpart", bufs=2)
            nc.scalar.activation(cp7, psA[3][:, 128:256], AF.Sigmoid)
            pt7 = psump.tile([128, B], fp32, name="pt7", tag="pt", bufs=2)
            nc.tensor.transpose(pt7, cp7, ident_sb[:, :])
            cur7T_sb = curp.tile([128, B], fp32, name="cur7T_sb", tag="c7T")
            nc.vector.tensor_copy(cur7T_sb, pt7)

            pso = psump.tile([64, 512], fp32, name="pso", tag="pso")
            nc.tensor.matmul(
                pso, lhsT=ones_sb[:, :], rhs=boutrow_sb[:, :], start=True, stop=False
            )
            nc.tensor.matmul(
                pso, lhsT=cur7T_sb, rhs=wout_sb[:, :], start=False, stop=True
            )
            out_sb = curp.tile([64, 512], fp32, name="out_sb", tag="out_sb")
            nc.vector.tensor_copy(out_sb, pso)
            rs_in = dramp.tile([64, 512], fp32, name="rs_in", tag="rs_in")
            rs_out = dramp.tile([8, 512], fp32, name="rs_out", tag="rs_out")
            nc.scalar.dma_start(out=rs_in, in_=out_sb)
            nc.gpsimd.collective_compute(
                "ReduceScatter",
                mybir.AluOpType.add,
                replica_groups=rg,
                ins=[rs_in.opt()],
                outs=[rs_out.opt()],
            )
            nc.scalar.dma_start(out=t_out[:, :], in_=rs_out)

    nc.compile()
    return nc


def _get_nc():
    global _BUILT
    if _BUILT is None:
        _BUILT = _build()
    return _BUILT


def make_in_maps(x, hidden_states, W_in, b_in, W_hid, b_hid, W_rec, W_skip,
                 W_out, b_out, mask_hid, mask_rec, mask_skip):
    x = np.asarray(x, np.float32)
    h = np.asarray(hidden_states, np.float32)
    W_in = np.asarray(W_in, np.float32)
    b_in = np.asarray(b_in, np.float32)
    W_out = np.asarray(W_out, np.float32)
    b_out = np.asarray(b_out, np.float32)
    Wh = np.asarray(W_hid, np.float32) * np.asarray(mask_hid, np.float32)
    Wr = np.asarray(W_rec, np.float32) * np.asarray(mask_rec, np.float32)
    Ws = np.asarray(W_skip, np.float32) * np.asarray(mask_skip, np.float32)
    b_hid = np.asarray(b_hid, np.float32)

    hT = _pack(h)
    xT = _pack(x)
    ones = np.ones((1, B), np.float32)
    ident = np.eye(B, dtype=np.float32)
    # every core's partial includes the bias and ReduceScatter sums them
    boutrow = np.ascontiguousarray(b_out[None, :]) / NCORES

    in_maps = []
    for c_ in range(NCORES):
        R = slice(c_ * RP, (c_ + 1) * RP)
        biases = [b_in[R]] + [b_hid[i, R] for i in range(7)]
        biasrow = np.zeros((1, 4 * 256), np.float32)
        for j in range(8):
            biasrow[0, j * 128:(j + 1) * 128] = biases[j]
        m = {
            "hT": hT,
            "xT": xT,
            "win": _pack(W_in[R]),
            "hid": np.stack([_pack(Wh[i, R]) for i in range(7)]),
            "biasrow": biasrow,
            "wout": np.ascontiguousarray(W_out[:, R].T),
            "boutrow": boutrow,
            "ones": ones,
            "ident": ident,
        }
        for j in range(7):
            m[f"rec{j}"] = _pack(Wr[j, R])
        packs = {j: _pack(Ws[j - 2, R, :j * W]).reshape(128, j * 8, 128)
                 for j in range(2, 8)}
        for q in range(1, 4):
            a = 2 * q
            for c in range(a + 1):
                if c < a:
                    Pa = packs[a][:, c * 8:(c + 1) * 8, :].reshape(128, 8 * 128)
                    Pb = packs[a + 1][:, c * 8:(c + 1) * 8, :].reshape(128, 8 * 128)
                    m[f"skip{q}_{c}"] = _interleave(
                        np.ascontiguousarray(Pa), np.ascontiguousarray(Pb)
                    )
                else:
                    m[f"skip{q}_{c}"] = np.ascontiguousarray(
                        packs[a + 1][:, c * 8:(c + 1) * 8, :].reshape(128, 8 * 128)
                    )
        in_maps.append(m)
    return in_maps


def run(in_maps, **kw):
    from concourse import bass_utils
    nc = _get_nc()
    return bass_utils.run_bass_kernel_spmd(
        nc, in_maps, core_ids=list(range(NCORES)), **kw
    )


def kernel(**inputs):
    in_maps = make_in_maps(**inputs)
    res = run(in_maps)
    return np.ascontiguousarray(
        np.concatenate([res.results[c]["out"] for c in range(NCORES)], axis=0),
        dtype=np.float32,
    )



# revision 7
# speedup vs baseline: 1.5543x; 1.5543x over previous
"""BrainRNN forward pass on 8 TRN2 NeuronCores (Bass/Tile, SPMD).

Strategy (tensor-parallel over output neurons, fp8/bf16 quantized):
  - Each block's 1024 output neurons are row-sharded 128/core; masks are
    folded into weights on the host.  Weights are streamed as fp8-e3m4
    scaled by S=128 (4x less HBM than fp32; verified ~4.5e-3 absmax err
    vs the 2e-2 gate, including a subnormal-flush worst case).
  - Matmuls run WEIGHTS-STATIONARY: lhsT = weight chunk [128k, 128m]
    (fp8, FWL-eligible), rhs = activation chunk [128k, 64b] (bf16).
    PSUM output is [128 neurons, 64 batch] -- the exact layout the
    AllGather needs, so the per-block PE-transpose + DVE copy of the
    fp32 version is gone, and each MM streams only 64 columns.
  - PSUM: blocks are paired 2-per-bank ([128, 128] fp32 in a full-bank
    [128,512] tile).  Each pair's accumulation group is opened by ONE
    K=2 matmul (biascol-pair x dual-ones) because start=True clears
    has_written for the whole bank.
  - Biases are pre-scaled by S on the host; the sigmoid applies the
    1/S descale via the ACT scale operand.
  - Chain per block: sigmoid (PSUM->SBUF bf16) -> gpsimd bounce DMA ->
    AllGather (bf16) -> 2-engine staggered unpack into curT [128, 8*64].
  - DMA emission order tracks deadlines; matmul emission back-fills
    AllGather wait windows with ready work (rec of later blocks, skip
    contributions from earlier sources).
  - Output block: AllGather cur7, then each core computes only its own
    64 output columns (out = cur7 @ W_out[:, C].T + b_out[C]); host
    concatenates.  No ReduceScatter.
"""

import numpy as np
import ml_dtypes

N = 8192
W = 1024
L = 8
B = 64
IN = 512
OUT = 512
NCORES = 8
RP = W // NCORES        # 128 rows per core per block
OC = OUT // NCORES      # 64 output columns per core
S = 128.0               # fp8 weight pre-scale (power of two; descaled at sigmoid)

E3M4 = ml_dtypes.float8_e3m4
BF16 = ml_dtypes.bfloat16

_BUILT = None
DEBUG_DUMP = False          # adds per-block curT dumps as external outputs


def _pack(A):
    """[M, K] -> [128, (K/128)*M] with packed[p, k*M+m] = A[m, k*128+p].

    Chunk kidx is A[:, kidx*128:(kidx+1)*128].T, i.e. [K=128(part), M(free)].
    """
    M, K = A.shape
    nk = K // 128
    return np.ascontiguousarray(
        A.reshape(M, nk, 128).transpose(2, 1, 0).reshape(128, nk * M)
    )


def _q8(A):
    """fp32 weights -> e3m4 with S pre-scale (saturating clip)."""
    return np.clip(np.asarray(A, np.float32) * S, -15.5, 15.5).astype(E3M4)


def _build():
    import concourse.bass as bass
    import concourse.bacc as bacc
    import concourse.mybir as mybir
    import concourse.tile as tile

    fp32 = mybir.dt.float32
    bf16 = mybir.dt.bfloat16
    fp8 = mybir.dt.float8e3
    AF = mybir.ActivationFunctionType

    nc = bacc.Bacc(
        "TRN2",
        target_bir_lowering=False,
        debug=False,
        enable_asserts=False,
        num_devices=NCORES,
    )

    t_hT = nc.dram_tensor("hT", [128, 64 * B], bf16, kind="ExternalInput")
    t_xT = nc.dram_tensor("xT", [128, 4 * B], bf16, kind="ExternalInput")
    t_win = nc.dram_tensor("win", [128, 4 * RP], fp8, kind="ExternalInput")
    t_rec = {
        j: nc.dram_tensor(f"rec{j}", [128, 64 * RP], fp8, kind="ExternalInput")
        for j in range(7)
    }
    t_hid = {
        j: nc.dram_tensor(f"hid{j}", [128, 8 * RP], fp8, kind="ExternalInput")
        for j in range(1, 8)
    }
    t_skip = {
        j: nc.dram_tensor(f"skip{j}", [128, j * 8 * RP], fp8, kind="ExternalInput")
        for j in range(2, 8)
    }
    # pair-opener operands: per pair q, biascolpair[2, 128] (row s = bias of
    # block 2q+s) and onesdual[2, 128] ([ones|zeros] / [zeros|ones]).
    t_biascol = nc.dram_tensor("biascol", [2, 4 * RP], fp32, kind="ExternalInput")
    t_onesdual = nc.dram_tensor("onesdual", [2, 2 * B], fp32, kind="ExternalInput")
    t_ones = nc.dram_tensor("ones", [1, B], fp32, kind="ExternalInput")
    t_boutcol = nc.dram_tensor("boutcol", [1, OC], fp32, kind="ExternalInput")
    t_wout = nc.dram_tensor("wout", [128, 8 * OC], bf16, kind="ExternalInput")
    t_out = nc.dram_tensor("out", [B, OC], fp32, kind="ExternalOutput")
    t_dbg = {
        j: nc.dram_tensor(f"dbg{j}", [128, 8 * B], bf16, kind="ExternalOutput")
        for j in range(8)
    } if DEBUG_DUMP else {}

    rg = [list(range(NCORES))]
    qof = lambda j: j // 2
    side = lambda j: j % 2

    with tile.TileContext(nc) as tc:
        with (
            tc.tile_pool(name="const", bufs=1) as constp,
            tc.tile_pool(name="wgt", bufs=1) as wgtp,
            tc.tile_pool(name="curs", bufs=1) as curp,
            tc.tile_pool(name="psum", bufs=1, space="PSUM") as psump,
            tc.tile_pool(name="dram", bufs=1, space="DRAM") as dramp,
        ):
            # ---- persistent inputs -------------------------------------
            biascol_sb = constp.tile([2, 4 * RP], fp32, name="biascol_sb", tag="bc")
            nc.sync.dma_start(out=biascol_sb, in_=t_biascol[:, :])
            onesdual_sb = constp.tile([2, 2 * B], fp32, name="onesdual_sb", tag="od")
            nc.sync.dma_start(out=onesdual_sb, in_=t_onesdual[:, :])
            ones_sb = constp.tile([1, B], fp32, name="ones_sb", tag="ones")
            nc.sync.dma_start(out=ones_sb, in_=t_ones[:, :])
            boutcol_sb = constp.tile([1, OC], fp32, name="boutcol_sb", tag="bo")
            nc.sync.dma_start(out=boutcol_sb, in_=t_boutcol[:, :])

            hT_sb = constp.tile([128, 64 * B], bf16, name="hT_sb", tag="hT")
            for i in range(2):
                nc.sync.dma_start(
                    out=hT_sb[:, i * 32 * B:(i + 1) * 32 * B],
                    in_=t_hT[:, i * 32 * B:(i + 1) * 32 * B],
                )
            xT_sb = constp.tile([128, 4 * B], bf16, name="xT_sb", tag="xT")
            nc.sync.dma_start(out=xT_sb, in_=t_xT[:, :])
            win_sb = constp.tile([128, 4 * RP], fp8, name="win_sb", tag="win")
            nc.sync.dma_start(out=win_sb, in_=t_win[:, :])

            # ---- PSUM: 4 pair banks + 1 out bank -----------------------
            # full-bank [128, 512] tiles force bank exclusivity; pair q uses
            # cols 0:128 (block 2q at 0:64, block 2q+1 at 64:128).
            psP = [
                psump.tile([128, 512], fp32, name=f"psP{q}", tag=f"psP{q}")
                for q in range(4)
            ]
            psO = psump.tile([64, 512], fp32, name="psO", tag="psO")

            def blk_ps(j):
                q, s = qof(j), side(j)
                return psP[q][:, s * B:(s + 1) * B]

            # pair openers: K=2 outer product writes both blocks' bias and
            # clears the bank's has_written exactly once.
            for q in range(4):
                nc.tensor.matmul(
                    psP[q][:, 0:2 * B],
                    lhsT=biascol_sb[:, q * RP:(q + 1) * RP],
                    rhs=onesdual_sb[:, :],
                    start=True,
                    stop=False,
                )

            curT = [None] * 8
            rec_sb = {}
            hid_sb = {}
            skip_sb = {}

            def rec_dma(j, nsplit=2):
                rt = wgtp.tile([128, 64 * RP], fp8, name=f"rec{j}sb", tag=f"rec{j}")
                step = 64 * RP // nsplit
                for i in range(nsplit):
                    nc.sync.dma_start(
                        out=rt[:, i * step:(i + 1) * step],
                        in_=t_rec[j][:, i * step:(i + 1) * step],
                    )
                rec_sb[j] = rt

            def rec_mms(j):
                """h @ Wrec[j].T accumulated into block j's psum region."""
                rt = rec_sb[j]
                ps = blk_ps(j)
                for kk in range(64):
                    nc.tensor.matmul(
                        ps,
                        lhsT=rt[:, kk * RP:(kk + 1) * RP],
                        rhs=hT_sb[:, kk * B:(kk + 1) * B],
                        start=False,
                        stop=False,
                    )

            def hid_dma(j):
                ht = wgtp.tile([128, 8 * RP], fp8, name=f"hid{j}sb", tag=f"hid{j}")
                nc.sync.dma_start(out=ht, in_=t_hid[j][:, :])
                hid_sb[j] = ht

            def skip_dma(j, nsplit=1):
                st = wgtp.tile(
                    [128, j * 8 * RP], fp8, name=f"skip{j}sb", tag=f"skip{j}"
                )
                step = j * 8 * RP // nsplit
                for i in range(nsplit):
                    nc.sync.dma_start(
                        out=st[:, i * step:(i + 1) * step],
                        in_=t_skip[j][:, i * step:(i + 1) * step],
                    )
                skip_sb[j] = st

            def hid_mms(j, stop=False):
                """cur_{j-1} @ W_hid[j-1].T into block j's psum region."""
                ht = hid_sb[j]
                ps = blk_ps(j)
                for kk in range(8):
                    nc.tensor.matmul(
                        ps,
                        lhsT=ht[:, kk * RP:(kk + 1) * RP],
                        rhs=curT[j - 1][:, kk * B:(kk + 1) * B],
                        start=False,
                        stop=(stop and kk == 7),
                    )

            def skip_mms(j, c, stop=False):
                """skips[src block c] @ W_skip into block j's psum region."""
                st = skip_sb[j]
                ps = blk_ps(j)
                for kk in range(8):
                    ck = c * 8 + kk
                    nc.tensor.matmul(
                        ps,
                        lhsT=st[:, ck * RP:(ck + 1) * RP],
                        rhs=curT[c][:, kk * B:(kk + 1) * B],
                        start=False,
                        stop=(stop and kk == 7),
                    )

            def chain_tail(j):
                """sigmoid (with 1/S descale) -> bounce -> AllGather -> unpack."""
                cs = curp.tile([128, B], bf16, name=f"cs{j}", tag="cs", bufs=2)
                nc.scalar.activation(cs, blk_ps(j), AF.Sigmoid, scale=1.0 / S)
                agin = dramp.tile([128, B], bf16, name=f"agin{j}", tag=f"agin{j}")
                agout = dramp.tile([W, B], bf16, name=f"agout{j}", tag=f"agout{j}")
                nc.gpsimd.dma_start(out=agin, in_=cs)
                nc.gpsimd.collective_compute(
                    "AllGather",
                    mybir.AluOpType.bypass,
                    replica_groups=rg,
                    ins=[agin.opt()],
                    outs=[agout.opt()],
                )
                return agout

            def unpack(j, agout):
                dst = curp.tile([128, 8 * B], bf16, name=f"curT{j}", tag=f"curT{j}")
                # gpsimd just observed the collective's completion sem; scalar
                # handles the other half on its own ring.
                for eng, kk in ((nc.gpsimd, 0), (nc.scalar, 4)):
                    eng.dma_start(
                        out=dst[:, kk * B:(kk + 4) * B].rearrange(
                            "p (k b) -> p k b", k=4
                        ),
                        in_=agout[kk * 128:(kk + 4) * 128, :].rearrange(
                            "(k p) b -> p k b", p=128
                        ),
                    )
                curT[j] = dst
                if DEBUG_DUMP:
                    nc.sync.dma_start(out=t_dbg[j][:, :], in_=dst)

            # ---- block 0: x @ W_in.T + h @ Wrec0.T (+bias) -------------
            rec_dma(0, nsplit=4)
            for kk in range(4):
                nc.tensor.matmul(
                    blk_ps(0),
                    lhsT=win_sb[:, kk * RP:(kk + 1) * RP],
                    rhs=xT_sb[:, kk * B:(kk + 1) * B],
                    start=False,
                    stop=False,
                )
            rec_mms(0)
            ag0 = chain_tail(0)

            # AG0 window
            rec_dma(1)
            rec_mms(1)
            hid_dma(1)
            unpack(0, ag0)
            hid_mms(1, stop=True)       # pair 0 (blocks 0,1) complete
            ag1 = chain_tail(1)

            # AG1 window
            rec_dma(2)
            rec_mms(2)
            hid_dma(2)
            skip_dma(2)
            unpack(1, ag1)
            skip_mms(2, 0)
            skip_mms(2, 1)
            hid_mms(2)
            ag2 = chain_tail(2)

            # AG2 window
            rec_dma(3)
            rec_mms(3)
            hid_dma(3)
            skip_dma(3)
            skip_mms(3, 0)
            skip_mms(3, 1)
            unpack(2, ag2)
            skip_mms(3, 2)
            hid_mms(3, stop=True)       # pair 1 (blocks 2,3) complete
            ag3 = chain_tail(3)

            # AG3 window
            rec_dma(4)
            rec_mms(4)
            hid_dma(4)
            skip_dma(4)
            skip_mms(4, 0)
            skip_mms(4, 1)
            skip_mms(4, 2)
            unpack(3, ag3)
            skip_mms(4, 3)
            hid_mms(4)
            ag4 = chain_tail(4)

            # AG4 window
            rec_dma(5)
            rec_mms(5)
            hid_dma(5)
            skip_dma(5)
            skip_mms(5, 0)
            skip_mms(5, 1)
            skip_mms(5, 2)
            skip_mms(5, 3)
            unpack(4, ag4)
            skip_mms(5, 4)
            hid_mms(5, stop=True)       # pair 2 (blocks 4,5) complete
            ag5 = chain_tail(5)

            # AG5 window
            rec_dma(6)
            rec_mms(6)
            hid_dma(6)
            skip_dma(6)
            for c in range(5):
                skip_mms(6, c)
            unpack(5, ag5)
            skip_mms(6, 5)
            hid_mms(6)
            ag6 = chain_tail(6)

            # AG6 window
            hid_dma(7)
            skip_dma(7)
            wout_sb = constp.tile([128, 8 * OC], bf16, name="wout_sb", tag="wout")
            nc.sync.dma_start(out=wout_sb, in_=t_wout[:, :])
            for c in range(6):
                skip_mms(7, c)
            unpack(6, ag6)
            skip_mms(7, 6)
            hid_mms(7, stop=True)       # pair 3 (blocks 6,7) complete
            ag7 = chain_tail(7)

            # AG7 window: open the output accumulator
            nc.tensor.matmul(
                psO[:, 0:OC], lhsT=boutcol_sb, rhs=ones_sb, start=True, stop=False
            )
            unpack(7, ag7)
            for kk in range(8):
                nc.tensor.matmul(
                    psO[:, 0:OC],
                    lhsT=wout_sb[:, kk * OC:(kk + 1) * OC],
                    rhs=curT[7][:, kk * B:(kk + 1) * B],
                    start=False,
                    stop=(kk == 7),
                )
            out_sb = curp.tile([B, OC], fp32, name="out_sb", tag="out_sb")
            nc.vector.tensor_copy(out_sb, psO[:, 0:OC])
            nc.scalar.dma_start(out=t_out[:, :], in_=out_sb)

    nc.compile()
    return nc


def _get_nc():
    global _BUILT
    if _BUILT is None:
        _BUILT = _build()
    return _BUILT


def make_in_maps(x, hidden_states, W_in, b_in, W_hid, b_hid, W_rec, W_skip,
                 W_out, b_out, mask_hid, mask_rec, mask_skip):
    x = np.asarray(x, np.float32)
    h = np.asarray(hidden_states, np.float32)
    W_in = np.asarray(W_in, np.float32)
    b_in = np.asarray(b_in, np.float32)
    W_out = np.asarray(W_out, np.float32)
    b_out = np.asarray(b_out, np.float32)
    Wh = np.asarray(W_hid, np.float32) * np.asarray(mask_hid, np.float32)
    Wr = np.asarray(W_rec, np.float32) * np.asarray(mask_rec, np.float32)
    Ws = np.asarray(W_skip, np.float32) * np.asarray(mask_skip, np.float32)
    b_hid = np.asarray(b_hid, np.float32)

    hT = _pack(h.astype(BF16))
    xT = _pack(x.astype(BF16))
    ones = np.ones((1, B), np.float32)
    onesdual = np.zeros((2, 2 * B), np.float32)
    onesdual[0, :B] = 1.0
    onesdual[1, B:] = 1.0

    in_maps = []
    for c_ in range(NCORES):
        R = slice(c_ * RP, (c_ + 1) * RP)
        C = slice(c_ * OC, (c_ + 1) * OC)
        biases = [b_in[R] * S] + [b_hid[i, R] * S for i in range(7)]
        biascol = np.zeros((2, 4 * RP), np.float32)
        for j in range(8):
            biascol[j % 2, (j // 2) * RP:(j // 2 + 1) * RP] = biases[j]
        m = {
            "hT": hT,
            "xT": xT,
            "win": _pack(_q8(W_in[R])),
            "biascol": biascol,
            "onesdual": onesdual,
            "ones": ones,
            "boutcol": np.ascontiguousarray(b_out[None, C]),
            "wout": _pack(W_out[C].astype(BF16)),
        }
        for j in range(7):
            m[f"rec{j}"] = _pack(_q8(Wr[j, R]))
        for j in range(1, 8):
            m[f"hid{j}"] = _pack(_q8(Wh[j - 1, R]))
        for j in range(2, 8):
            m[f"skip{j}"] = _pack(_q8(Ws[j - 2, R, :j * W]))
        in_maps.append(m)
    return in_maps


def run(in_maps, **kw):
    from concourse import bass_utils
    nc = _get_nc()
    return bass_utils.run_bass_kernel_spmd(
        nc, in_maps, core_ids=list(range(NCORES)), **kw
    )


def kernel(**inputs):
    in_maps = make_in_maps(**inputs)
    res = run(in_maps)
    return np.ascontiguousarray(
        np.concatenate([res.results[c]["out"].T for c in range(NCORES)], axis=1),
        dtype=np.float32,
    )


# revision 15
# speedup vs baseline: 1.6924x; 1.0889x over previous
"""BrainRNN forward pass on 8 TRN2 NeuronCores (Bass/Tile, SPMD).

Strategy (tensor-parallel over output neurons, fp8/bf16 quantized):
  - Each block's 1024 output neurons are row-sharded 128/core; masks are
    folded into weights on the host.  Weights are streamed as fp8-e3m4
    scaled by S=128 (4x less HBM than fp32; verified ~4.5e-3 absmax err
    vs the 2e-2 gate, including a subnormal-flush worst case).
  - Matmuls run WEIGHTS-STATIONARY: lhsT = weight chunk [128k, 128m]
    (fp8, FWL-eligible), rhs = activation chunk [128k, 64b] (bf16).
    PSUM output is [128 neurons, 64 batch] -- the exact layout the
    AllGather needs, so the per-block PE-transpose + DVE copy of the
    fp32 version is gone, and each MM streams only 64 columns.
  - PSUM: blocks are paired 2-per-bank ([128, 128] fp32 in a full-bank
    [128,512] tile).  Each pair's accumulation group is opened by ONE
    K=2 matmul (biascol-pair x dual-ones) because start=True clears
    has_written for the whole bank.
  - Biases are pre-scaled by S on the host; the sigmoid applies the
    1/S descale via the ACT scale operand.
  - Chain per block: sigmoid (PSUM->SBUF bf16) -> gpsimd bounce DMA ->
    AllGather (bf16) -> 2-engine staggered unpack into curT [128, 8*64].
  - DMA emission order tracks deadlines; matmul emission back-fills
    AllGather wait windows with ready work (rec of later blocks, skip
    contributions from earlier sources).
  - Output block: AllGather cur7, then each core computes only its own
    64 output columns (out = cur7 @ W_out[:, C].T + b_out[C]); host
    concatenates.  No ReduceScatter.
"""

import numpy as np
import ml_dtypes

N = 8192
W = 1024
L = 8
B = 64
IN = 512
OUT = 512
NCORES = 8
RP = W // NCORES        # 128 rows per core per block
OC = OUT // NCORES      # 64 output columns per core
S = 128.0               # fp8 weight pre-scale (power of two; descaled at sigmoid)

E3M4 = ml_dtypes.float8_e3m4
BF16 = ml_dtypes.bfloat16

_BUILT = None


def _pack(A):
    """[M, K] -> [128, (K/128)*M] with packed[p, k*M+m] = A[m, k*128+p].

    Chunk kidx is A[:, kidx*128:(kidx+1)*128].T, i.e. [K=128(part), M(free)].
    """
    M, K = A.shape
    nk = K // 128
    return np.ascontiguousarray(
        A.reshape(M, nk, 128).transpose(2, 1, 0).reshape(128, nk * M)
    )


def _q8(A):
    """fp32 weights -> e3m4 with S pre-scale (saturating clip)."""
    return np.clip(np.asarray(A, np.float32) * S, -15.5, 15.5).astype(E3M4)


def _build():
    import concourse.bass as bass
    import concourse.bacc as bacc
    import concourse.mybir as mybir
    import concourse.tile as tile

    fp32 = mybir.dt.float32
    bf16 = mybir.dt.bfloat16
    fp8 = mybir.dt.float8e3
    AF = mybir.ActivationFunctionType

    nc = bacc.Bacc(
        "TRN2",
        target_bir_lowering=False,
        debug=False,
        enable_asserts=False,
        num_devices=NCORES,
    )

    t_hT = nc.dram_tensor("hT", [128, 64 * B], bf16, kind="ExternalInput")
    t_xT = nc.dram_tensor("xT", [128, 4 * B], bf16, kind="ExternalInput")
    t_win = nc.dram_tensor("win", [128, 4 * RP], fp8, kind="ExternalInput")
    t_rec = {
        j: nc.dram_tensor(f"rec{j}", [128, 64 * RP], fp8, kind="ExternalInput")
        for j in range(7)
    }
    t_hid = {
        j: nc.dram_tensor(f"hid{j}", [128, 8 * RP], fp8, kind="ExternalInput")
        for j in range(1, 8)
    }
    t_skip = {
        j: nc.dram_tensor(f"skip{j}", [128, j * 8 * RP], fp8, kind="ExternalInput")
        for j in range(2, 8)
    }
    t_biascol = nc.dram_tensor("biascol", [2, 4 * RP], fp32, kind="ExternalInput")
    t_onesdual = nc.dram_tensor("onesdual", [2, 2 * B], fp32, kind="ExternalInput")
    t_ones = nc.dram_tensor("ones", [1, B], fp32, kind="ExternalInput")
    t_boutcol = nc.dram_tensor("boutcol", [1, OC], fp32, kind="ExternalInput")
    t_wout = nc.dram_tensor("wout", [128, 8 * OC], bf16, kind="ExternalInput")
    t_out = nc.dram_tensor("out", [OC, B], fp32, kind="ExternalOutput")

    rg = [list(range(NCORES))]
    qof = lambda j: j // 2
    side = lambda j: j % 2

    with tile.TileContext(nc) as tc:
        with (
            tc.tile_pool(name="const", bufs=1) as constp,
            tc.tile_pool(name="wgt", bufs=1) as wgtp,
            tc.tile_pool(name="curs", bufs=1) as curp,
            tc.tile_pool(name="psum", bufs=1, space="PSUM") as psump,
            tc.tile_pool(name="dram", bufs=1, space="DRAM") as dramp,
        ):
            # ---- persistent inputs -------------------------------------
            biascol_sb = constp.tile([2, 4 * RP], fp32, name="biascol_sb", tag="bc")
            nc.sync.dma_start(out=biascol_sb, in_=t_biascol[:, :])
            onesdual_sb = constp.tile([2, 2 * B], fp32, name="onesdual_sb", tag="od")
            nc.sync.dma_start(out=onesdual_sb, in_=t_onesdual[:, :])
            ones_sb = constp.tile([1, B], fp32, name="ones_sb", tag="ones")
            nc.sync.dma_start(out=ones_sb, in_=t_ones[:, :])
            boutcol_sb = constp.tile([1, OC], fp32, name="boutcol_sb", tag="bo")
            nc.sync.dma_start(out=boutcol_sb, in_=t_boutcol[:, :])

            hT_sb = constp.tile([128, 64 * B], bf16, name="hT_sb", tag="hT")
            for i in range(2):
                nc.sync.dma_start(
                    out=hT_sb[:, i * 32 * B:(i + 1) * 32 * B],
                    in_=t_hT[:, i * 32 * B:(i + 1) * 32 * B],
                )
            xT_sb = constp.tile([128, 4 * B], bf16, name="xT_sb", tag="xT")
            nc.sync.dma_start(out=xT_sb, in_=t_xT[:, :])
            win_sb = constp.tile([128, 4 * RP], fp8, name="win_sb", tag="win")
            nc.sync.dma_start(out=win_sb, in_=t_win[:, :])

            # ---- PSUM: 4 pair banks + 1 out bank -----------------------
            psP = [
                psump.tile([128, 512], fp32, name=f"psP{q}", tag=f"psP{q}")
                for q in range(4)
            ]
            psO = psump.tile([64, 512], fp32, name="psO", tag="psO")

            def blk_ps(j):
                q, s = qof(j), side(j)
                return psP[q][:, s * B:(s + 1) * B]

            # pair openers: K=2 outer product writes both blocks' bias and
            # clears the bank's has_written exactly once.
            for q in range(4):
                nc.tensor.matmul(
                    psP[q][:, 0:2 * B],
                    lhsT=biascol_sb[:, q * RP:(q + 1) * RP],
                    rhs=onesdual_sb[:, :],
                    start=True,
                    stop=False,
                )

            curT = [None] * 8
            rec_sb = {}
            hid_sb = {}
            skip_sb = {}

            def rec_dma(j, nsplit=2):
                rt = wgtp.tile([128, 64 * RP], fp8, name=f"rec{j}sb", tag=f"rec{j}")
                step = 64 * RP // nsplit
                for i in range(nsplit):
                    nc.sync.dma_start(
                        out=rt[:, i * step:(i + 1) * step],
                        in_=t_rec[j][:, i * step:(i + 1) * step],
                    )
                rec_sb[j] = rt

            def rec_mms(j):
                rt = rec_sb[j]
                ps = blk_ps(j)
                for kk in range(64):
                    nc.tensor.matmul(
                        ps,
                        lhsT=rt[:, kk * RP:(kk + 1) * RP],
                        rhs=hT_sb[:, kk * B:(kk + 1) * B],
                        start=False,
                        stop=False,
                    )

            def hid_dma(j):
                ht = wgtp.tile([128, 8 * RP], fp8, name=f"hid{j}sb", tag=f"hid{j}")
                nc.sync.dma_start(out=ht, in_=t_hid[j][:, :])
                hid_sb[j] = ht

            def skip_dma(j):
                st = wgtp.tile(
                    [128, j * 8 * RP], fp8, name=f"skip{j}sb", tag=f"skip{j}"
                )
                nc.sync.dma_start(out=st, in_=t_skip[j][:, :])
                skip_sb[j] = st

            def hid_mms(j, stop=False):
                ht = hid_sb[j]
                ps = blk_ps(j)
                for kk in range(8):
                    nc.tensor.matmul(
                        ps,
                        lhsT=ht[:, kk * RP:(kk + 1) * RP],
                        rhs=curT[j - 1][:, kk * B:(kk + 1) * B],
                        start=False,
                        stop=(stop and kk == 7),
                    )

            def skip_mms(j, c):
                st = skip_sb[j]
                ps = blk_ps(j)
                for kk in range(8):
                    ck = c * 8 + kk
                    nc.tensor.matmul(
                        ps,
                        lhsT=st[:, ck * RP:(ck + 1) * RP],
                        rhs=curT[c][:, kk * B:(kk + 1) * B],
                        start=False,
                        stop=False,
                    )

            def chain_tail(j):
                cs = curp.tile([128, B], bf16, name=f"cs{j}", tag="cs", bufs=2)
                nc.scalar.activation(cs, blk_ps(j), AF.Sigmoid, scale=1.0 / S)
                agin = dramp.tile([128, B], bf16, name=f"agin{j}", tag=f"agin{j}")
                agout = dramp.tile([W, B], bf16, name=f"agout{j}", tag=f"agout{j}")
                nc.gpsimd.dma_start(out=agin, in_=cs)
                nc.gpsimd.collective_compute(
                    "AllGather",
                    mybir.AluOpType.bypass,
                    replica_groups=rg,
                    ins=[agin.opt()],
                    outs=[agout.opt()],
                )
                return agout

            def unpack(j, agout):
                dst = curp.tile([128, 8 * B], bf16, name=f"curT{j}", tag=f"curT{j}")
                for eng, kk in ((nc.gpsimd, 0), (nc.scalar, 4)):
                    eng.dma_start(
                        out=dst[:, kk * B:(kk + 4) * B].rearrange(
                            "p (k b) -> p k b", k=4
                        ),
                        in_=agout[kk * 128:(kk + 4) * 128, :].rearrange(
                            "(k p) b -> p k b", p=128
                        ),
                    )
                curT[j] = dst

            # ---- block 0: x @ W_in.T + h @ Wrec0.T (+bias) -------------
            rec_dma(0, nsplit=4)
            for kk in range(4):
                nc.tensor.matmul(
                    blk_ps(0),
                    lhsT=win_sb[:, kk * RP:(kk + 1) * RP],
                    rhs=xT_sb[:, kk * B:(kk + 1) * B],
                    start=False,
                    stop=False,
                )
            rec_mms(0)
            ag0 = chain_tail(0)

            # AG0 window
            rec_dma(1)
            rec_mms(1)
            hid_dma(1)
            unpack(0, ag0)
            hid_mms(1, stop=True)       # pair 0 (blocks 0,1) complete
            ag1 = chain_tail(1)

            # AG1 window
            rec_dma(2)
            rec_mms(2)
            hid_dma(2)
            skip_dma(2)
            unpack(1, ag1)
            skip_mms(2, 0)
            skip_mms(2, 1)
            hid_mms(2)
            ag2 = chain_tail(2)

            # AG2 window
            rec_dma(3)
            rec_mms(3)
            hid_dma(3)
            skip_dma(3)
            skip_mms(3, 0)
            skip_mms(3, 1)
            unpack(2, ag2)
            skip_mms(3, 2)
            hid_mms(3, stop=True)       # pair 1 (blocks 2,3) complete
            ag3 = chain_tail(3)

            # AG3 window
            rec_dma(4)
            rec_mms(4)
            hid_dma(4)
            skip_dma(4)
            skip_mms(4, 0)
            skip_mms(4, 1)
            skip_mms(4, 2)
            unpack(3, ag3)
            skip_mms(4, 3)
            hid_mms(4)
            ag4 = chain_tail(4)

            # AG4 window
            rec_dma(5)
            rec_mms(5)
            hid_dma(5)
            skip_dma(5)
            skip_mms(5, 0)
            skip_mms(5, 1)
            skip_mms(5, 2)
            skip_mms(5, 3)
            unpack(4, ag4)
            skip_mms(5, 4)
            hid_mms(5, stop=True)       # pair 2 (blocks 4,5) complete
            ag5 = chain_tail(5)

            # AG5 window
            rec_dma(6)
            rec_mms(6)
            hid_dma(6)
            skip_dma(6)
            for c in range(5):
                skip_mms(6, c)
            unpack(5, ag5)
            skip_mms(6, 5)
            hid_mms(6)
            ag6 = chain_tail(6)

            # AG6 window
            hid_dma(7)
            skip_dma(7)
            wout_sb = constp.tile([128, 8 * OC], bf16, name="wout_sb", tag="wout")
            nc.sync.dma_start(out=wout_sb, in_=t_wout[:, :])
            for c in range(6):
                skip_mms(7, c)
            unpack(6, ag6)
            skip_mms(7, 6)
            hid_mms(7, stop=True)       # pair 3 (blocks 6,7) complete
            ag7 = chain_tail(7)

            # AG7 window: open the output accumulator
            nc.tensor.matmul(
                psO[:, 0:B], lhsT=boutcol_sb, rhs=ones_sb, start=True, stop=False
            )
            unpack(7, ag7)
            for kk in range(8):
                nc.tensor.matmul(
                    psO[:, 0:B],
                    lhsT=wout_sb[:, kk * OC:(kk + 1) * OC],
                    rhs=curT[7][:, kk * B:(kk + 1) * B],
                    start=False,
                    stop=(kk == 7),
                )
            out_sb = curp.tile([OC, B], fp32, name="out_sb", tag="out_sb")
            nc.vector.tensor_copy(out_sb, psO[:, 0:B])
            nc.scalar.dma_start(out=t_out[:, :], in_=out_sb)

    nc.compile()
    return nc


def _get_nc():
    global _BUILT
    if _BUILT is None:
        _BUILT = _build()
    return _BUILT


def make_in_maps(x, hidden_states, W_in, b_in, W_hid, b_hid, W_rec, W_skip,
                 W_out, b_out, mask_hid, mask_rec, mask_skip):
    x = np.asarray(x, np.float32)
    h = np.asarray(hidden_states, np.float32)
    W_in = np.asarray(W_in, np.float32)
    b_in = np.asarray(b_in, np.float32)
    W_out = np.asarray(W_out, np.float32)
    b_out = np.asarray(b_out, np.float32)
    Wh = np.asarray(W_hid, np.float32) * np.asarray(mask_hid, np.float32)
    Wr = np.asarray(W_rec, np.float32) * np.asarray(mask_rec, np.float32)
    Ws = np.asarray(W_skip, np.float32) * np.asarray(mask_skip, np.float32)
    b_hid = np.asarray(b_hid, np.float32)

    hT = _pack(h.astype(BF16))
    xT = _pack(x.astype(BF16))
    ones = np.ones((1, B), np.float32)
    onesdual = np.zeros((2, 2 * B), np.float32)
    onesdual[0, :B] = 1.0
    onesdual[1, B:] = 1.0

    in_maps = []
    for c_ in range(NCORES):
        R = slice(c_ * RP, (c_ + 1) * RP)
        C = slice(c_ * OC, (c_ + 1) * OC)
        biases = [b_in[R] * S] + [b_hid[i, R] * S for i in range(7)]
        biascol = np.zeros((2, 4 * RP), np.float32)
        for j in range(8):
            biascol[j % 2, (j // 2) * RP:(j // 2 + 1) * RP] = biases[j]
        m = {
            "hT": hT,
            "xT": xT,
            "win": _pack(_q8(W_in[R])),
            "biascol": biascol,
            "onesdual": onesdual,
            "ones": ones,
            "boutcol": np.ascontiguousarray(b_out[None, C]),
            "wout": _pack(W_out[C].astype(BF16)),
        }
        for j in range(7):
            m[f"rec{j}"] = _pack(_q8(Wr[j, R]))
        for j in range(1, 8):
            m[f"hid{j}"] = _pack(_q8(Wh[j - 1, R]))
        for j in range(2, 8):
            m[f"skip{j}"] = _pack(_q8(Ws[j - 2, R, :j * W]))
        in_maps.append(m)
    return in_maps


def run(in_maps, **kw):
    from concourse import bass_utils
    nc = _get_nc()
    return bass_utils.run_bass_kernel_spmd(
        nc, in_maps, core_ids=list(range(NCORES)), **kw
    )


def kernel(**inputs):
    in_maps = make_in_maps(**inputs)
    res = run(in_maps)
    return np.ascontiguousarray(
        np.concatenate([res.results[c]["out"].T for c in range(NCORES)], axis=1),
        dtype=np.float32,
    )
